# revision 1
# baseline (speedup 1.0000x reference)
"""BiLSTM-CRF loss kernel for 8 Trainium2 NeuronCores.

Sharding: phase 1 (embedding-projection + LSTM + emission GEMM) runs on
8 cores = 2 directions x 4 batch-quarters (16 examples/core, 512 steps).
The backward direction is realized by feeding time-reversed embeddings
through the same forward program. Phase 2 (CRF forward algorithm) runs
on 8 cores = 8 examples/core. Host glues the phases (pair-sum of the
two fc halves, gold-path score gathers, final logsumexp/mean).
"""

import numpy as np
import ml_dtypes

import concourse.bacc as bacc
import concourse.mybir as mybir
from concourse import tile
from concourse.bass_utils import run_bass_kernel_spmd

V, T, E, H = 50000, 32, 256, 512
B, S = 64, 512
BC = 16            # batch per core, phase 1
B2 = 8             # batch per core, phase 2
NCORES = 8
CHUNK = 32         # LSTM steps per projection chunk
NCHUNK = S // CHUNK
GMAP = [0, 1, 3, 2]   # psum gate-block order i,f,o,g -> weight-row gate index
RENORM = 8         # CRF renormalization cadence

AF = mybir.ActivationFunctionType
F32 = mybir.dt.float32
BF16 = mybir.dt.bfloat16
ALU = mybir.AluOpType

_built = {}


def _new_nc():
    return bacc.Bacc("TRN2", target_bir_lowering=False, debug=False,
                     num_devices=NCORES)


def build_phase1(nsteps=S):
    nc = _new_nc()
    nch = nsteps // CHUNK
    eT = nc.dram_tensor("eT", [2, 128, nsteps * BC], BF16, kind="ExternalInput")
    wih = nc.dram_tensor("wihT", [2, 128, 4 * H], BF16, kind="ExternalInput")
    whh = nc.dram_tensor("whhT", [4, 128, 4 * H], BF16, kind="ExternalInput")
    fcw = nc.dram_tensor("fcwT", [4, 128, T], BF16, kind="ExternalInput")
    bia = nc.dram_tensor("biasT", [128, 16], F32, kind="ExternalInput")
    emo = nc.dram_tensor("emT", [T, nsteps * BC], F32, kind="ExternalOutput")

    with tile.TileContext(nc) as tc:
        with (
            tc.tile_pool(name="weights", bufs=1) as wpool,
            tc.tile_pool(name="state", bufs=1) as spool,
            tc.tile_pool(name="et", bufs=2) as epool,
            tc.tile_pool(name="xp", bufs=2) as xpool,
            tc.tile_pool(name="gact", bufs=2) as apool,
            tc.tile_pool(name="tmp", bufs=2) as tpool,
            tc.tile_pool(name="psg", bufs=2, space="PSUM") as pgpool,
            tc.tile_pool(name="psp", bufs=2, space="PSUM") as pppool,
            tc.tile_pool(name="pse", bufs=2, space="PSUM") as pepool,
        ):
            wih_s = wpool.tile([128, 2, 4 * H], BF16, tag="wih")
            whh_s = wpool.tile([128, 4, 4 * H], BF16, tag="whh")
            fcw_s = wpool.tile([128, 4, T], BF16, tag="fcw")
            bia_s = wpool.tile([128, 16], F32, tag="bias")
            hbuf = spool.tile([128, 4, nsteps * BC], BF16, tag="hbuf")
            czero = spool.tile([128, 4, BC], F32, tag="cstate")
            hzero = spool.tile([128, BC], BF16, tag="hzero")

            for k in range(2):
                nc.gpsimd.dma_start(wih_s[:, k, :], wih[k, :, :])
            for k in range(4):
                nc.gpsimd.dma_start(whh_s[:, k, :], whh[k, :, :])
                nc.gpsimd.dma_start(fcw_s[:, k, :], fcw[k, :, :])
            nc.gpsimd.dma_start(bia_s[:], bia[:, :])
            nc.vector.memset(czero[:], 0.0)
            nc.vector.memset(hzero[:], 0.0)

            ct = czero  # persistent cell state, updated in place

            def project(c, xp_t):
                et = epool.tile([128, 2, CHUNK * BC], BF16, tag="et")
                cs = slice(c * CHUNK * BC, (c + 1) * CHUNK * BC)
                for k in range(2):
                    nc.gpsimd.dma_start(et[:, k, :], eT[k, :, cs])
                for gg in range(4):
                    for k in range(4):
                        m = GMAP[gg] * 4 + k
                        pp = pppool.tile([128, CHUNK * BC], F32, tag="psp")
                        for ek in range(2):
                            nc.tensor.matmul(
                                pp[:], wih_s[:, ek, m * 128:(m + 1) * 128],
                                et[:, ek, :], start=(ek == 0), stop=(ek == 1))
                        nc.scalar.activation(xp_t[:, gg, k, :], pp[:],
                                             AF.Identity, bias=bia_s[:, m:m + 1])

            xp_tiles = []
            for c in range(min(2, nch)):
                xp_t = xpool.tile([128, 4, 4, CHUNK * BC], F32, tag="xp")
                project(c, xp_t)
                xp_tiles.append(xp_t)

            for t in range(nsteps):
                c, tt = divmod(t, CHUNK)
                xp_t = xp_tiles[c % 2]
                ps = pgpool.tile([128, 4, 4, BC], F32, tag="psg")
                for gg in range(4):
                    for k in range(4):
                        m = GMAP[gg] * 4 + k
                        for hk in range(4):
                            rhs = (hzero[:] if t == 0 else
                                   hbuf[:, hk, (t - 1) * BC:t * BC])
                            nc.tensor.matmul(
                                ps[:, gg, k, :],
                                whh_s[:, hk, m * 128:(m + 1) * 128], rhs,
                                start=(hk == 0), stop=(hk == 3))
                gsum = tpool.tile([128, 4, 4, BC], F32, tag="gsum")
                nc.vector.tensor_add(gsum[:], ps[:],
                                     xp_t[:, :, :, tt * BC:(tt + 1) * BC])
                gact = apool.tile([128, 4, 4, BC], F32, tag="gact")
                nc.scalar.activation(gact[:, 0:3], gsum[:, 0:3], AF.Sigmoid)
                nc.scalar.activation(gact[:, 3], gsum[:, 3], AF.Tanh)
                itg = tpool.tile([128, 4, BC], F32, tag="itg")
                nc.vector.tensor_mul(itg[:], gact[:, 0], gact[:, 3])
                nc.vector.tensor_mul(ct[:], gact[:, 1], ct[:])
                nc.vector.tensor_add(ct[:], ct[:], itg[:])
                tch = tpool.tile([128, 4, BC], F32, tag="tch")
                nc.scalar.activation(tch[:], ct[:], AF.Tanh)
                nc.vector.tensor_mul(hbuf[:, :, t * BC:(t + 1) * BC],
                                     gact[:, 2], tch[:])
                if tt == CHUNK - 1 and c + 2 < nch:
                    project(c + 2, xp_tiles[c % 2])

            for ch in range(nch):
                pe = pepool.tile([T, CHUNK * BC], F32, tag="pse")
                cs = slice(ch * CHUNK * BC, (ch + 1) * CHUNK * BC)
                for hk in range(4):
                    nc.tensor.matmul(pe[:], fcw_s[:, hk, :], hbuf[:, hk, cs],
                                     start=(hk == 0), stop=(hk == 3))
                est = tpool.tile([T, CHUNK * BC], F32, tag="est")
                nc.scalar.copy(est[:], pe[:])
                nc.gpsimd.dma_start(emo[:, cs], est[:])
    nc.compile()
    return nc


def build_phase2(nsteps=S):
    nc = _new_nc()
    em = nc.dram_tensor("emT2", [T, nsteps * B2], F32, kind="ExternalInput")
    ex = nc.dram_tensor("expT", [T, T + 1], F32, kind="ExternalInput")
    ao = nc.dram_tensor("alphaO", [T, B2], F32, kind="ExternalOutput")
    zo = nc.dram_tensor("zO", [1, B2], F32, kind="ExternalOutput")

    with tile.TileContext(nc) as tc:
        with (
            tc.tile_pool(name="sb", bufs=1) as sb,
            tc.tile_pool(name="lp", bufs=2) as lp,
            tc.tile_pool(name="ps", bufs=2, space="PSUM") as pp,
            tc.tile_pool(name="psb", bufs=2, space="PSUM") as pb,
        ):
            em_s = sb.tile([T, nsteps * B2], F32, tag="em")
            ex_s = sb.tile([T, T + 1], F32, tag="ex")
            ones = sb.tile([1, T], F32, tag="ones")
            alpha = sb.tile([T, B2], F32, tag="alpha")
            zacc = sb.tile([1, B2], F32, tag="z")
            nc.gpsimd.dma_start(em_s[:], em[:, :])
            nc.gpsimd.dma_start(ex_s[:], ex[:, :])
            nc.vector.memset(ones[:], 1.0)
            nc.vector.memset(zacc[:], 0.0)
            nc.scalar.copy(alpha[:], em_s[:, 0:B2])

            for t in range(1, nsteps):
                ea = lp.tile([T, B2], F32, tag="ea")
                nc.scalar.activation(ea[:], alpha[:], AF.Exp)
                pt = pp.tile([T + 1, B2], F32, tag="pt")
                nc.tensor.matmul(pt[:], ex_s[:], ea[:], start=True, stop=True)
                lg = lp.tile([T, B2], F32, tag="lg")
                nc.scalar.activation(lg[:], pt[0:T, :], AF.Ln)
                es = slice(t * B2, (t + 1) * B2)
                if t % RENORM == RENORM - 1:
                    lg32 = lp.tile([1, B2], F32, tag="lg32")
                    nc.scalar.activation(lg32[:], pt[T:T + 1, :], AF.Ln)
                    bc = pb.tile([T, B2], F32, tag="bc")
                    nc.tensor.matmul(bc[:], ones[:], lg32[:],
                                     start=True, stop=True)
                    nc.vector.tensor_add(alpha[:], lg[:], em_s[:, es])
                    nc.vector.tensor_sub(alpha[:], alpha[:], bc[:])
                    nc.vector.tensor_add(zacc[:], zacc[:], lg32[:])
                else:
                    nc.vector.tensor_add(alpha[:], lg[:], em_s[:, es])

            nc.gpsimd.dma_start(ao[:, :], alpha[:])
            nc.gpsimd.dma_start(zo[:, :], zacc[:])
    nc.compile()
    return nc


def _bf16(a):
    return np.ascontiguousarray(a.astype(ml_dtypes.bfloat16))


def _prep_core_p1(e_sbe, wih_d, whh_d, b_d, fcw_half):
    """e_sbe: [16, S, E] embedded (already time-reversed for bwd cores)."""
    eT = _bf16(e_sbe.transpose(2, 1, 0).reshape(2, 128, S * BC))
    wihT = _bf16(wih_d.T.reshape(2, 128, 4 * H))
    whhT = _bf16(whh_d.T.reshape(4, 128, 4 * H))
    fcwT = _bf16(fcw_half.T.reshape(4, 128, T))
    biasT = np.ascontiguousarray(
        b_d.reshape(16, 128).T.astype(np.float32))
    return {"eT": eT, "wihT": wihT, "whhT": whhT, "fcwT": fcwT,
            "biasT": biasT}


def kernel(emb, w_ih_f, w_hh_f, b_f, w_ih_b, w_hh_b, b_b, fc_w, fc_b,
           start_trans, end_trans, trans, x, tags):
    emb = np.asarray(emb, np.float32)
    fc_w = np.asarray(fc_w, np.float32)
    fc_b = np.asarray(fc_b, np.float32)
    start_trans = np.asarray(start_trans, np.float32)
    end_trans = np.asarray(end_trans, np.float32)
    trans = np.asarray(trans, np.float32)
    x = np.asarray(x).astype(np.int64)
    tags_np = np.asarray(tags).astype(np.int64)

    if "p1" not in _built:
        _built["p1"] = build_phase1()
        _built["p2"] = build_phase2()
    nc1, nc2 = _built["p1"], _built["p2"]

    in_maps = []
    for core in range(NCORES):
        d = core // 4          # 0 = forward, 1 = backward
        q = core % 4
        xs = x[q * BC:(q + 1) * BC]
        if d == 1:
            xs = xs[:, ::-1]
        e = emb[xs]            # [16, S, E]
        if d == 0:
            in_maps.append(_prep_core_p1(e, np.asarray(w_ih_f, np.float32),
                                         np.asarray(w_hh_f, np.float32),
                                         np.asarray(b_f, np.float32),
                                         fc_w[:, :H]))
        else:
            in_maps.append(_prep_core_p1(e, np.asarray(w_ih_b, np.float32),
                                         np.asarray(w_hh_b, np.float32),
                                         np.asarray(b_b, np.float32),
                                         fc_w[:, H:]))
    r1 = run_bass_kernel_spmd(nc1, in_maps, core_ids=list(range(NCORES)))

    em = np.empty((S, B, T), np.float32)
    for q in range(4):
        emf = r1.results[q]["emT"].reshape(T, S, BC).transpose(1, 2, 0)
        emb_r = r1.results[4 + q]["emT"].reshape(T, S, BC).transpose(1, 2, 0)
        em[:, q * BC:(q + 1) * BC, :] = emf + emb_r[::-1] + fc_b
    em[0] += start_trans

    # gold-path (numerator) score from device emissions + tag lookups
    tags_t = tags_np.T
    emit = np.take_along_axis(em, tags_t[:, :, None], axis=2)[..., 0].sum(0)
    tr = trans[tags_t[:-1], tags_t[1:]].sum(0)
    num = emit + tr + end_trans[tags_t[-1]]
    # (start_trans already folded into em[0])

    expT = np.concatenate([np.exp(trans), np.ones((T, 1), np.float32)],
                          axis=1).astype(np.float32)
    in_maps2 = []
    for core in range(NCORES):
        emc = em[:, core * B2:(core + 1) * B2, :]       # [S, 8, T]
        emT2 = np.ascontiguousarray(
            emc.transpose(2, 0, 1).reshape(T, S * B2).astype(np.float32))
        in_maps2.append({"emT2": emT2, "expT": expT})
    r2 = run_bass_kernel_spmd(nc2, in_maps2, core_ids=list(range(NCORES)))

    den = np.empty(B, np.float64)
    for core in range(NCORES):
        a = r2.results[core]["alphaO"].astype(np.float64)   # [T, 8]
        z = r2.results[core]["zO"][0].astype(np.float64)    # [8]
        a = a + end_trans[:, None]
        m = a.max(0)
        den[core * B2:(core + 1) * B2] = (
            np.log(np.exp(a - m).sum(0)) + m + z)

    llh = num - den
    return np.float32(-llh.mean())



# revision 5
# speedup vs baseline: 1.6014x; 1.6014x over previous
"""BiLSTM-CRF loss kernel for 8 Trainium2 NeuronCores.

Phase 1 (LSTM + emissions): 8 cores = 2 directions x 4 batch-quarters
(16 examples/core, 512 steps). The input projection (wih) and bias are
folded into the per-step PSUM accumulation: an ACT copy preloads the
bias two steps ahead, bf16 wih matmuls add the input projection on
top, and the recurrent whh matmuls accumulate last. The whh burst is
ordered g-gate-first so the gating chain (tanh g -> sigmoid f,i ->
cell update -> tanh c -> h) starts after 8 of the 64 matmuls and
overlaps the rest. Gating uses a paired layout (f,i | c,g~) so the
cell update is two DVE ops. Everything is bf16 except the f32 PSUM
accumulation.

Phase 2 (CRF partition function): exp-space linear recurrence
a' = (M^T a) * exp(em_t) -- one matmul + one DVE multiply per step, no
per-step exp/ln (the old log-space version reloaded the activation
table twice per step). Split into a forward chain over steps 0..255
and a backward chain over 511..256 run concurrently on each core
(8 examples/core), combined at the midpoint on the host. Renormalize
by the tag-sum every 8 steps (tracked in log space).
"""

import numpy as np
import ml_dtypes

import concourse.bacc as bacc
import concourse.mybir as mybir
from concourse import tile
from concourse.bass_utils import run_bass_kernel_spmd

V, T, E, H = 50000, 32, 256, 512
B, S = 64, 512
BC = 16            # batch per core, phase 1
B2 = 8             # batch per core, phase 2
NCORES = 8
CHUNK = 32         # steps per embedding-DMA / emission-GEMM chunk
RENORM = 8         # CRF renormalization cadence

AF = mybir.ActivationFunctionType
F32 = mybir.dt.float32
BF16 = mybir.dt.bfloat16
ALU = mybir.AluOpType

# psum gate-block order g,f,i,o ; PyTorch row order is i,f,g,o
GPERM = np.r_[1024:1536, 512:1024, 0:512, 1536:2048]

_built = {}


def _new_nc():
    return bacc.Bacc("TRN2", target_bir_lowering=False, debug=False,
                     num_devices=NCORES)


def build_phase1(nsteps=S):
    nc = _new_nc()
    nch = nsteps // CHUNK
    eb = nc.dram_tensor("eb", [2, 128, nsteps * BC], BF16,
                        kind="ExternalInput")
    wih = nc.dram_tensor("wihb", [128, 2, 4 * H], BF16, kind="ExternalInput")
    whh = nc.dram_tensor("whhb", [128, 4, 4 * H], BF16, kind="ExternalInput")
    fcw = nc.dram_tensor("fcwb", [128, 4, T], BF16, kind="ExternalInput")
    bbc = nc.dram_tensor("biasbc", [128, 16, BC], F32, kind="ExternalInput")
    emo = nc.dram_tensor("emT", [T, nsteps * BC], F32, kind="ExternalOutput")

    with tile.TileContext(nc) as tc:
        with (
            tc.tile_pool(name="weights", bufs=1) as wpool,
            tc.tile_pool(name="state", bufs=1) as spool,
            tc.tile_pool(name="et", bufs=2) as epool,
            tc.tile_pool(name="gfi", bufs=2) as apool,
            tc.tile_pool(name="go", bufs=2) as opool,
            tc.tile_pool(name="pp", bufs=2) as ppool,
            tc.tile_pool(name="tch", bufs=2) as tpool,
            tc.tile_pool(name="est", bufs=2) as espool,
            tc.tile_pool(name="psg", bufs=3, space="PSUM") as pgpool,
            tc.tile_pool(name="pse", bufs=2, space="PSUM") as pepool,
        ):
            wih_s = wpool.tile([128, 2, 4 * H], BF16, tag="wih")
            whh_s = wpool.tile([128, 4, 4 * H], BF16, tag="whh")
            fcw_s = wpool.tile([128, 4, T], BF16, tag="fcw")
            bbc_s = wpool.tile([128, 16, BC], F32, tag="bbc")
            hbuf = spool.tile([128, 4, nsteps * BC], BF16, tag="hbuf")
            cg = spool.tile([128, 8, BC], BF16, tag="cg")  # [c | g~]

            for k in range(2):
                nc.gpsimd.dma_start(wih_s[:, k, :], wih[:, k, :])
            for k in range(4):
                nc.gpsimd.dma_start(whh_s[:, k, :], whh[:, k, :])
                nc.gpsimd.dma_start(fcw_s[:, k, :], fcw[:, k, :])
            nc.gpsimd.dma_start(bbc_s[:], bbc[:, :, :])
            nc.vector.memset(cg[:, 0:4, :], 0.0)

            def et_dma(ch):
                etile = epool.tile([128, 2, CHUNK * BC], BF16, tag="et")
                cs = slice(ch * CHUNK * BC, (ch + 1) * CHUNK * BC)
                for k in range(2):
                    nc.gpsimd.dma_start(etile[:, k, :], eb[k, :, cs])
                return etile

            et_tiles = [et_dma(0), et_dma(1)]

            def bias_copy(t):
                ps = pgpool.tile([128, 16, BC], F32, tag="psg")
                nc.scalar.copy(ps[:], bbc_s[:])
                return ps

            def wih_mm(t, ps):
                et = et_tiles[(t // CHUNK) % 2]
                es = slice((t % CHUNK) * BC, (t % CHUNK + 1) * BC)
                for m in range(16):
                    for k in range(2):
                        nc.tensor.matmul(
                            ps[:, m, :],
                            wih_s[:, k, m * 128:(m + 1) * 128],
                            et[:, k, es], start=False, stop=False,
                            skip_group_check=True)

            ps_tiles = [bias_copy(0), bias_copy(1)]
            wih_mm(0, ps_tiles[0])
            wih_mm(1, ps_tiles[1])

            for t in range(nsteps):
                ps = ps_tiles[t % 2]
                ch, tt = divmod(t, CHUNK)
                if t > 0:
                    hs = slice((t - 1) * BC, t * BC)
                    for m in range(16):       # g blocks first, o last
                        for j in range(4):
                            nc.tensor.matmul(
                                ps[:, m, :],
                                whh_s[:, j, m * 128:(m + 1) * 128],
                                hbuf[:, j, hs],
                                start=False, stop=False,
                                skip_group_check=True)
                if t + 2 < nsteps:
                    ps_next = bias_copy(t + 2)   # ACT, top of step
                else:
                    ps_next = None
                nc.scalar.activation(cg[:, 4:8, :], ps[:, 0:4, :], AF.Tanh)
                gfi = apool.tile([128, 8, BC], BF16, tag="gfi")
                nc.scalar.activation(gfi[:], ps[:, 4:12, :], AF.Sigmoid)
                go = opool.tile([128, 4, BC], BF16, tag="go")
                nc.scalar.activation(go[:], ps[:, 12:16, :], AF.Sigmoid)
                pp = ppool.tile([128, 8, BC], BF16, tag="pp")
                nc.vector.tensor_mul(pp[:], gfi[:], cg[:])
                nc.vector.tensor_add(cg[:, 0:4, :], pp[:, 0:4, :],
                                     pp[:, 4:8, :])
                tch = tpool.tile([128, 4, BC], BF16, tag="tch")
                nc.scalar.activation(tch[:], cg[:, 0:4, :], AF.Tanh)
                nc.vector.tensor_mul(hbuf[:, :, t * BC:(t + 1) * BC],
                                     go[:], tch[:])
                if ps_next is not None:
                    wih_mm(t + 2, ps_next)       # PE, after whh(t)
                    ps_tiles[t % 2] = ps_next
                if tt == CHUNK - 1:
                    cs = slice(ch * CHUNK * BC, (ch + 1) * CHUNK * BC)
                    pe = pepool.tile([T, CHUNK * BC], F32, tag="pse")
                    for j in range(4):
                        nc.tensor.matmul(pe[:], fcw_s[:, j, :],
                                         hbuf[:, j, cs],
                                         start=(j == 0), stop=(j == 3))
                    est = espool.tile([T, CHUNK * BC], F32, tag="est")
                    nc.vector.tensor_scalar_add(est[:], pe[:], 0.0)
                    nc.gpsimd.dma_start(emo[:, cs], est[:])
                if tt == CHUNK - 2 and ch + 2 < nch:
                    et_tiles[ch % 2] = et_dma(ch + 2)
    nc.compile()
    return nc


def build_phase2(nsteps=S, mid=None):
    if mid is None:
        mid = nsteps // 2 - 1
    nc = _new_nc()
    nf = nsteps * B2
    em = nc.dram_tensor("emS", [T, nf], F32, kind="ExternalInput")
    mfw = nc.dram_tensor("mfw", [T, T + 1], BF16, kind="ExternalInput")
    mbw = nc.dram_tensor("mbw", [T, T + 1], BF16, kind="ExternalInput")
    u0d = nc.dram_tensor("u0", [T, B2], BF16, kind="ExternalInput")
    aO = nc.dram_tensor("aO", [T, B2], BF16, kind="ExternalOutput")
    bO = nc.dram_tensor("bO", [T, B2], BF16, kind="ExternalOutput")
    zaO = nc.dram_tensor("zaO", [1, B2], F32, kind="ExternalOutput")
    zbO = nc.dram_tensor("zbO", [1, B2], F32, kind="ExternalOutput")

    with tile.TileContext(nc) as tc:
        with (
            tc.tile_pool(name="sb", bufs=1) as sb,
            tc.tile_pool(name="ab", bufs=3) as ab,
            tc.tile_pool(name="rr", bufs=2) as rr,
            tc.tile_pool(name="pf", bufs=2, space="PSUM") as pf,
            tc.tile_pool(name="pb", bufs=2, space="PSUM") as pb,
            tc.tile_pool(name="pr", bufs=2, space="PSUM") as pr,
        ):
            em_s = sb.tile([T, nf], F32, tag="em")
            emx = sb.tile([T, nf], BF16, tag="emx")
            mf_s = sb.tile([T, T + 1], BF16, tag="mf")
            mb_s = sb.tile([T, T + 1], BF16, tag="mb")
            onesT = sb.tile([1, T], F32, tag="ones")
            u0_s = sb.tile([T, B2], BF16, tag="u0")
            za = sb.tile([1, B2], F32, tag="za")
            zb = sb.tile([1, B2], F32, tag="zb")
            nc.gpsimd.dma_start(em_s[:], em[:, :])
            nc.gpsimd.dma_start(mf_s[:], mfw[:, :])
            nc.gpsimd.dma_start(mb_s[:], mbw[:, :])
            nc.gpsimd.dma_start(u0_s[:], u0d[:, :])
            nc.vector.memset(onesT[:], 1.0)
            nc.vector.memset(za[:], 0.0)
            nc.vector.memset(zb[:], 0.0)
            nc.scalar.activation(emx[:], em_s[:], AF.Exp)

            def exslice(t):
                return emx[:, t * B2:(t + 1) * B2]

            a = ab.tile([T, B2], BF16, tag="a")
            nc.vector.tensor_scalar_add(a[:], exslice(0), 0.0)
            u = ab.tile([T, B2], BF16, tag="u")
            nc.vector.tensor_mul(u[:], u0_s[:], exslice(nsteps - 1))

            for i in range(nsteps - 1 - mid):
                tf = 1 + i
                kb = nsteps - 1 - i
                if tf <= mid:
                    pt = pf.tile([T + 1, B2], F32, tag="ptf")
                    nc.tensor.matmul(pt[:], mf_s[:], a[:],
                                     start=True, stop=True)
                    a2 = ab.tile([T, B2], BF16, tag="a")
                    nc.vector.tensor_mul(a2[:], pt[0:T, :], exslice(tf))
                    a = a2
                    if tf % RENORM == RENORM - 1:
                        rec = rr.tile([1, B2], F32, tag="rf")
                        nc.vector.reciprocal(rec[:], pt[T:T + 1, :])
                        rb = pr.tile([T, B2], F32, tag="rbf")
                        nc.tensor.matmul(rb[:], onesT[:], rec[:],
                                         start=True, stop=True)
                        a3 = ab.tile([T, B2], BF16, tag="a")
                        nc.vector.tensor_mul(a3[:], a2[:], rb[:])
                        a = a3
                        lg = rr.tile([1, B2], F32, tag="lf")
                        nc.scalar.activation(lg[:], pt[T:T + 1, :], AF.Ln)
                        nc.vector.tensor_add(za[:], za[:], lg[:])
                ptb = pb.tile([T + 1, B2], F32, tag="ptb")
                nc.tensor.matmul(ptb[:], mb_s[:], u[:], start=True, stop=True)
                if kb - 1 > mid:
                    u2 = ab.tile([T, B2], BF16, tag="u")
                    nc.vector.tensor_mul(u2[:], ptb[0:T, :], exslice(kb - 1))
                    u = u2
                    if kb % RENORM == 0:
                        recb = rr.tile([1, B2], F32, tag="rb")
                        nc.vector.reciprocal(recb[:], ptb[T:T + 1, :])
                        rbb = pr.tile([T, B2], F32, tag="rbb")
                        nc.tensor.matmul(rbb[:], onesT[:], recb[:],
                                         start=True, stop=True)
                        u3 = ab.tile([T, B2], BF16, tag="u")
                        nc.vector.tensor_mul(u3[:], u2[:], rbb[:])
                        u = u3
                        lgb = rr.tile([1, B2], F32, tag="lb")
                        nc.scalar.activation(lgb[:], ptb[T:T + 1, :], AF.Ln)
                        nc.vector.tensor_add(zb[:], zb[:], lgb[:])
                else:
                    bout = sb.tile([T, B2], BF16, tag="bout")
                    nc.vector.tensor_scalar_add(bout[:], ptb[0:T, :], 0.0)
                    nc.gpsimd.dma_start(bO[:, :], bout[:])
            nc.gpsimd.dma_start(aO[:, :], a[:])
            nc.gpsimd.dma_start(zaO[:, :], za[:])
            nc.gpsimd.dma_start(zbO[:, :], zb[:])
    nc.compile()
    return nc


def _bf16(a):
    return np.ascontiguousarray(np.asarray(a).astype(ml_dtypes.bfloat16))


def _prep_core_p1(e_core, wih_d, whh_d, b_d, fcw_half):
    """e_core: [16, ns, E] bf16 embeddings (already reversed for bwd)."""
    ns = e_core.shape[1]
    eT = np.ascontiguousarray(
        e_core.transpose(2, 1, 0).reshape(2, 128, ns * BC))
    wp = wih_d[GPERM]                       # [4H, E]
    wihb = np.ascontiguousarray(
        wp.T.reshape(2, 128, 4 * H).transpose(1, 0, 2).astype(
            ml_dtypes.bfloat16))
    hp = whh_d[GPERM]                       # [4H, H]
    whhb = np.ascontiguousarray(
        hp.T.reshape(4, 128, 4 * H).transpose(1, 0, 2).astype(
            ml_dtypes.bfloat16))
    fcwb = np.ascontiguousarray(
        fcw_half.T.reshape(4, 128, T).transpose(1, 0, 2).astype(
            ml_dtypes.bfloat16))
    bp = b_d[GPERM].reshape(16, 128).T      # [128, 16]
    biasbc = np.ascontiguousarray(
        np.repeat(bp[:, :, None], BC, axis=2).astype(np.float32))
    return {"eb": eT, "wihb": wihb, "whhb": whhb, "fcwb": fcwb,
            "biasbc": biasbc}


def kernel(emb, w_ih_f, w_hh_f, b_f, w_ih_b, w_hh_b, b_b, fc_w, fc_b,
           start_trans, end_trans, trans, x, tags):
    emb = np.asarray(emb, np.float32)
    fc_w = np.asarray(fc_w, np.float32)
    fc_b = np.asarray(fc_b, np.float32)
    start_trans = np.asarray(start_trans, np.float32)
    end_trans = np.asarray(end_trans, np.float32)
    trans = np.asarray(trans, np.float32)
    x = np.asarray(x).astype(np.int64)
    tags_np = np.asarray(tags).astype(np.int64)

    if "p1" not in _built:
        _built["p1"] = build_phase1()
        _built["p2"] = build_phase2()
    nc1, nc2 = _built["p1"], _built["p2"]

    embb = emb.astype(ml_dtypes.bfloat16)
    in_maps = []
    for core in range(NCORES):
        d = core // 4          # 0 = forward, 1 = backward
        q = core % 4
        xs = x[q * BC:(q + 1) * BC]
        if d == 1:
            xs = xs[:, ::-1]
        ec = embb[xs]          # [16, S, E] bf16
        if d == 0:
            in_maps.append(_prep_core_p1(
                ec, np.asarray(w_ih_f, np.float32),
                np.asarray(w_hh_f, np.float32),
                np.asarray(b_f, np.float32), fc_w[:, :H]))
        else:
            in_maps.append(_prep_core_p1(
                ec, np.asarray(w_ih_b, np.float32),
                np.asarray(w_hh_b, np.float32),
                np.asarray(b_b, np.float32), fc_w[:, H:]))
    r1 = run_bass_kernel_spmd(nc1, in_maps, core_ids=list(range(NCORES)))

    em = np.empty((S, B, T), np.float32)
    for q in range(4):
        emf = r1.results[q]["emT"].reshape(T, S, BC).transpose(1, 2, 0)
        emb_r = r1.results[4 + q]["emT"].reshape(T, S, BC).transpose(1, 2, 0)
        em[:, q * BC:(q + 1) * BC, :] = emf + emb_r[::-1] + fc_b
    em[0] += start_trans

    # gold-path (numerator) score; start_trans already folded into em[0]
    tags_t = tags_np.T
    emit = np.take_along_axis(em, tags_t[:, :, None], axis=2)[..., 0].sum(0)
    tr = trans[tags_t[:-1], tags_t[1:]].sum(0)
    num = emit + tr + end_trans[tags_t[-1]]

    mfw = np.concatenate([np.exp(trans), np.ones((T, 1), np.float32)], axis=1)
    mbw = np.concatenate([np.exp(trans).T, np.ones((T, 1), np.float32)],
                         axis=1)
    u0 = np.repeat(np.exp(end_trans)[:, None], B2, axis=1)
    in_maps2 = []
    for core in range(NCORES):
        emc = em[:, core * B2:(core + 1) * B2, :]           # [S, 8, T]
        emS = np.ascontiguousarray(
            emc.transpose(2, 0, 1).reshape(T, S * B2).astype(np.float32))
        in_maps2.append({"emS": emS, "mfw": _bf16(mfw), "mbw": _bf16(mbw),
                         "u0": _bf16(u0)})
    r2 = run_bass_kernel_spmd(nc2, in_maps2, core_ids=list(range(NCORES)))

    den = np.empty(B, np.float64)
    for core in range(NCORES):
        a = r2.results[core]["aO"].astype(np.float64)       # [T, 8]
        bv = r2.results[core]["bO"].astype(np.float64)      # [T, 8]
        za = r2.results[core]["zaO"][0].astype(np.float64)  # [8]
        zb = r2.results[core]["zbO"][0].astype(np.float64)  # [8]
        den[core * B2:(core + 1) * B2] = (
            np.log((a * bv).sum(0)) + za + zb)

    llh = num - den
    return np.float32(-llh.mean())
